# revision 11
# baseline (speedup 1.0000x reference)
"""MoE routing kernel for 8 Trainium2 NeuronCores (Bass/Tile, SPMD).

Strategy (expert-parallel, matching the sharding hint):
  - Host computes the gate (softmax + top-2) and dispatches tokens: each of
    the 8 cores owns 2 of the 16 routed experts (slot 0 = a high-count
    expert, slot 1 = a low-count expert; per-slot capacities) and receives
    only the tokens routed to its experts.
  - The output layer (ow) folds into each expert's second matmul on the host
    (w2ot = w2[e].T @ ow.T), shrinking stage-2 work by W/OUT = 4x.
  - The shared expert is sharded over its intermediate dim (2048/8=256 rows
    per core); every core computes a partial for all 2048 tokens, with ow
    folded in.  Bias terms that commute with the output layer (b2, sb2, ob)
    are applied analytically on the host.
  - Stage-1 weights (w1/w3/sw1/sw3) are stored in fp8 e3m4 scaled by S=16;
    leaky-relu is positively homogeneous, so the descale folds into the
    host-prepared bias columns and w2ot -- zero extra device work.
  - Schedule: one global software-pipelined PE stream (expert0 m-tiles,
    expert1 m-tiles, shared chunks x m-tiles) with stage-2 matmuls trailing
    by LAG units.  Weight m-tiles stream on the SP DMA queue in consumption
    order; x/shared/bias/output DMAs ride the GpSimd queue so the ACT/DVE
    queues carry only compute.
"""
import sys

if "/opt/trn_rl_repo" not in sys.path:
    sys.path.insert(0, "/opt/trn_rl_repo")

import numpy as np
import ml_dtypes
import concourse.bass as bass
import concourse.tile as tile
from concourse import mybir
from concourse.bass_utils import run_bass_kernel_spmd

B = 2048
W = 512
E = 16
TOPK = 2
INTER = 1024
SH = 2048
OUT = 128
NCORES = 8
EPC = E // NCORES          # experts per core = 2
SHS = SH // NCORES         # shared-expert inter slice per core = 256
KW = W // 128              # k-tiles over W = 4
MI = INTER // 128          # m-tiles over INTER = 8
MS = SHS // 128            # m-tiles over shared slice = 2
F32 = mybir.dt.float32
F16 = mybir.dt.float16
E3 = mybir.dt.float8e3     # fp8 e3m4 (4 mantissa bits)
NP16 = np.float16
NPE3 = ml_dtypes.float8_e3m4

WDT = E3                   # stage-1 weight dtype knob (E3 or F16)
NPW = NPE3 if WDT == E3 else NP16
S = 16.0                   # fixed stage-1 weight scale (folds into bias/w2ot)
LAG = 2                    # stage-2 matmuls trail stage 1 by LAG units

TRACE = False
TRACE_KW = {}
LAST_RESULTS = None


def _legalize_waits(nc):
    """This container's walrus accepts at most 1 sync wait per instruction
    (2 for EventSemaphore).  Hoist excess waits emitted by the Tile
    scheduler into standalone EventSemaphore instructions."""
    for fn in nc.m.functions:
        for blk in fn.blocks:
            out = []
            changed = False
            for inst in blk.instructions:
                si = getattr(inst, "sync_info", None)
                waits = list(si.on_wait) if si is not None and si.on_wait else []
                cap = 2 if isinstance(inst, mybir.InstEventSemaphore) else 1
                if len(waits) > cap:
                    extra, keep = waits[:-cap], waits[-cap:]
                    for i in range(0, len(extra), 2):
                        out.append(mybir.InstEventSemaphore(
                            name=nc.get_next_instruction_name(),
                            engine=inst.engine,
                            ins=[], outs=[],
                            sync_info=mybir.SyncInfo(
                                on_wait=list(extra[i:i + 2]), on_update=[]),
                        ))
                    si.on_wait = keep
                    changed = True
                out.append(inst)
            if changed:
                blk.instructions = out


def _token_chunks(cap):
    chunks = []
    off = 0
    while off < cap:
        sz = min(512, cap - off)
        chunks.append((off, sz))
        off += sz
    return chunks


def _build_nc(cap0, cap1, legalize=True):
    """SPMD Bass program for per-slot token capacities (multiples of 16)."""
    nc = bass.Bass("TRN2", target_bir_lowering=False, debug=False)
    caps = (cap0, cap1)

    def din(name, f, dt):
        return nc.dram_tensor(name, [128, f], dt, kind="ExternalInput").ap()

    xg0 = din("xg0", KW * cap0, F16)     # slot-0 gathered tokens, k-blocks
    xg1 = din("xg1", KW * cap1, F16)
    # per (slot, m): [w1 m-tile k0..3 | w3 m-tile k0..3], each [128,128]
    w13 = din("w13", EPC * MI * 2 * KW * 128, WDT)
    w2ot = din("w2ot", EPC * MI * OUT, F16)
    xt = din("xt", KW * B, F16)          # x.T k-blocks (shared expert)
    sw13 = din("sw13", MS * 2 * KW * 128, WDT)
    sw2ot = din("sw2ot", MS * OUT, F16)
    bias = din("bias", EPC * 2 * MI + 2 * MS, F32)

    yr = nc.dram_tensor("yr", [128, cap0 + cap1], F16, kind="ExternalOutput").ap()
    zt = nc.dram_tensor("zt", [128, B], F16, kind="ExternalOutput").ap()

    LR = mybir.ActivationFunctionType.Lrelu

    EW = MI * 2 * KW * 128      # w13 cols per expert slot

    with tile.TileContext(nc) as tc:
        with tc.tile_pool(name="wts", bufs=1) as wts, \
             tc.tile_pool(name="work", bufs=3) as work, \
             tc.tile_pool(name="hts", bufs=4) as hts, \
             tc.tile_pool(name="outs", bufs=2) as outs, \
             tc.tile_pool(name="ps", bufs=2, space="PSUM") as ps:

            # ---- scalar (ACT) hw-dynamic queue: bias first (tiny) ----
            bias_t = wts.tile([128, bias.shape[1]], F32, tag="bias")
            nc.scalar.dma_start(bias_t[:], bias[:])

            # ---- SP hw-dynamic queue: slot-0 prologue (first-MM gate) ----
            xg_ts = [None, None]
            xg_ts[0] = wts.tile([128, KW * cap0], F16, tag="xg0", name="xg0t")
            nc.sync.dma_start(xg_ts[0][:], xg0[:])
            w13e = []
            for s in range(EPC):
                w13e.append(wts.tile([128, EW], WDT, tag=f"w13e{s}",
                                     name=f"w13e{s}t"))
            MW = 2 * KW * 128           # cols per m-tile
            nc.sync.dma_start(w13e[0][:, 0:2 * MW], w13[:, 0:2 * MW])
            nc.sync.dma_start(w13e[0][:, 2 * MW:EW], w13[:, 2 * MW:EW])
            w2ot_ts = []
            for s in range(EPC):
                w2ot_ts.append(wts.tile([128, MI * OUT], F16, tag=f"w2ot{s}",
                                        name=f"w2ot{s}t"))
            nc.sync.dma_start(w2ot_ts[0][:], w2ot[:, 0:MI * OUT])
            sw13_t = wts.tile([128, MS * 2 * KW * 128], WDT, tag="sw13")
            nc.sync.dma_start(sw13_t[:], sw13[:])
            sw2ot_t = wts.tile([128, MS * OUT], F16, tag="sw2ot")
            nc.sync.dma_start(sw2ot_t[:], sw2ot[:])

            xt_t = wts.tile([128, KW * B], F16, tag="xt")

            def emit_xt():     # 2MB, issued once slot-0 stream is underway
                nc.scalar.dma_start(xt_t[:], xt[:])

            def emit_slot1():  # slot-1 inputs, issued mid slot-0 compute
                xg_ts[1] = wts.tile([128, KW * cap1], F16, tag="xg1",
                                    name="xg1t")
                nc.sync.dma_start(xg_ts[1][:], xg1[:])
                nc.sync.dma_start(w13e[1][:], w13[:, EW:2 * EW])
                nc.sync.dma_start(w2ot_ts[1][:], w2ot[:, MI * OUT:2 * MI * OUT])

            def b_col(col):
                return bias_t[:, col:col + 1]

            # ---- PE warmup: dummy matmuls ramp the clock while DMAs land --
            wz = wts.tile([128, 256], F16, tag="wz")
            nc.gpsimd.memset(wz[:], 0.0)
            pwarm = ps.tile([128, 128], F32, tag="py", bufs=2, name="pwarm")
            for _ in range(12):
                nc.tensor.matmul(pwarm[:], wz[:, 0:128], wz[:, 128:256],
                                 start=True, stop=True)

            # ---- unified software-pipelined compute stream ----
            pend = []

            def emit_stage2(f):
                f()

            def unit_stage1(csz, lhs_tile, lhs_col0, rhs_tile, rhs_cols,
                            b1c, b3c):
                """Emit stage-1 matmuls + act/mul for one (unit, m-tile)."""
                p1 = ps.tile([128, csz], F32, tag="p1", bufs=2)
                p3 = ps.tile([128, csz], F32, tag="p3", bufs=2)
                for k in range(KW):
                    lhs = lhs_tile[:, lhs_col0 + k * 128:lhs_col0 + (k + 1) * 128]
                    rhs = rhs_tile[:, rhs_cols[k]:rhs_cols[k] + csz]
                    nc.tensor.matmul(p1[:], lhs, rhs,
                                     start=(k == 0), stop=(k == KW - 1))
                for k in range(KW):
                    lhs = lhs_tile[:, lhs_col0 + (KW + k) * 128:
                                   lhs_col0 + (KW + k + 1) * 128]
                    rhs = rhs_tile[:, rhs_cols[k]:rhs_cols[k] + csz]
                    nc.tensor.matmul(p3[:], lhs, rhs,
                                     start=(k == 0), stop=(k == KW - 1))
                a = work.tile([128, csz], F16, tag="act_a")
                nc.scalar.activation(a[:], p1[:], LR, bias=b1c, alpha=0.01)
                t3 = work.tile([128, csz], F16, tag="act_b")
                nc.vector.tensor_scalar_add(t3[:], p3[:], b3c)
                h = hts.tile([128, csz], F16, tag="h")
                nc.vector.tensor_mul(h[:], a[:], t3[:])
                return h

            # unit order: slot-0 expert, shared (small tail chunks), slot-1
            units = []
            for (c0, csz) in _token_chunks(cap0):
                for m in range(MI):
                    units.append(("r", 0, c0, csz, m))
            sh_chunks = [(0, 512), (512, 512), (1024, 512),
                         (1536, 256), (1792, 256)]
            for (c0, csz) in sh_chunks:
                for m in range(MS):
                    units.append(("s", 0, c0, csz, m))
            for (c0, csz) in _token_chunks(cap1):
                for m in range(MI):
                    units.append(("r", 1, c0, csz, m))

            hooks = {2: emit_xt, 4: emit_slot1}
            py_tiles = {}
            for ui, u in enumerate(units):
                if ui in hooks:
                    hooks[ui]()
                kind, s, c0, csz, m = u
                if kind == "r":
                    rhs_cols = [k * caps[s] + c0 for k in range(KW)]
                    h = unit_stage1(csz, w13e[s], m * 2 * KW * 128,
                                    xg_ts[s], rhs_cols,
                                    b_col(s * 2 * MI + m),
                                    b_col(s * 2 * MI + MI + m))
                    key = ("r", s, c0)
                    if key not in py_tiles:
                        py_tiles[key] = ps.tile([128, csz], F32, tag="py", bufs=2, name="py")
                    py = py_tiles[key]
                    w2t = w2ot_ts[s]
                    lhs2 = w2t[:, m * OUT:(m + 1) * OUT]
                    last = (m == MI - 1)

                    def mk(py=py, lhs2=lhs2, h=h, m=m, last=last, s=s,
                           c0=c0, csz=csz):
                        def f():
                            nc.tensor.matmul(py[:], lhs2, h[:],
                                             start=(m == 0), stop=last)
                            if last:
                                yo = outs.tile([128, csz], F16, tag="yo")
                                nc.vector.tensor_copy(yo[:], py[:])
                                off = s * cap0 + c0
                                nc.scalar.dma_start(
                                    yr[:, off:off + csz], yo[:])
                        return f
                    pend.append(mk())
                else:
                    rhs_cols = [k * B + c0 for k in range(KW)]
                    h = unit_stage1(csz, sw13_t, m * 2 * KW * 128,
                                    xt_t, rhs_cols,
                                    b_col(EPC * 2 * MI + m),
                                    b_col(EPC * 2 * MI + MS + m))
                    key = ("s", 0, c0)
                    if key not in py_tiles:
                        py_tiles[key] = ps.tile([128, csz], F32, tag="pz", bufs=2, name="pz")
                    pz = py_tiles[key]
                    lhs2 = sw2ot_t[:, m * OUT:(m + 1) * OUT]
                    last = (m == MS - 1)

                    def mk(pz=pz, lhs2=lhs2, h=h, m=m, last=last,
                           c0=c0, csz=csz):
                        def f():
                            nc.tensor.matmul(pz[:], lhs2, h[:],
                                             start=(m == 0), stop=last)
                            if last:
                                zo = outs.tile([128, csz], F16, tag="zo")
                                nc.vector.tensor_copy(zo[:], pz[:])
                                nc.scalar.dma_start(
                                    zt[:, c0:c0 + csz], zo[:])
                        return f
                    pend.append(mk())
                if len(pend) > LAG:
                    emit_stage2(pend.pop(0))
            for f in pend:
                emit_stage2(f)

    if legalize:
        _legalize_waits(nc)
    return nc


_NC_CACHE = {}


def _kblocks(mat):
    """[Ktot, F] -> [128, (Ktot/128)*F], col block k = mat[128k:128(k+1), :]."""
    ktot, f = mat.shape
    assert ktot % 128 == 0
    return np.ascontiguousarray(
        mat.reshape(ktot // 128, 128, f).transpose(1, 0, 2).reshape(128, -1))


def _pack_w13(w1e, w3e, mtiles):
    """w1e/w3e: [I, W] fp32 (already scaled).  Returns [128, mtiles*2*KW*128]
    with per-m layout [w1 k0..3 | w3 k0..3]."""
    blocks = []
    w1t = w1e.T.reshape(KW, 128, mtiles * 128)   # [k, 128, I]
    w3t = w3e.T.reshape(KW, 128, mtiles * 128)
    for m in range(mtiles):
        for wt in (w1t, w3t):
            for k in range(KW):
                blocks.append(wt[k][:, m * 128:(m + 1) * 128])
    return np.ascontiguousarray(np.concatenate(blocks, axis=1))


def prepare(x, task_id, gate_w, w1, b1, w2, b2, w3, b3,
            sw1, sb1, sw2, sb2, sw3, sb3, ow, ob):
    x = np.asarray(x, np.float32)
    f32 = lambda a: np.asarray(a, np.float32)
    gate_w, w1, b1, w2, b2, w3, b3 = map(f32, (gate_w, w1, b1, w2, b2, w3, b3))
    sw1, sb1, sw2, sb2, sw3, sb3, ow, ob = map(
        f32, (sw1, sb1, sw2, sb2, sw3, sb3, ow, ob))

    # ---- host gate: softmax + top-2 ----
    logits = x @ gate_w.T
    logits -= logits.max(axis=1, keepdims=True)
    ex = np.exp(logits)
    scores = ex / ex.sum(axis=1, keepdims=True)
    order = np.argsort(-scores, axis=1, kind="stable")[:, :TOPK]

    tok_lists = [np.nonzero((order == e).any(axis=1))[0] for e in range(E)]
    counts = np.array([len(t) for t in tok_lists])
    ranked = np.argsort(-counts, kind="stable")
    slot_experts = [list(ranked[:NCORES]), list(ranked[NCORES:][::-1])]
    rup = lambda n: max(128, -(-n // 16) * 16)
    cap0 = rup(max(counts[e] for e in slot_experts[0]))
    cap1 = rup(max(counts[e] for e in slot_experts[1]))

    key = (cap0, cap1)
    if key not in _NC_CACHE:
        _NC_CACHE[key] = _build_nc(cap0, cap1)
    nc = _NC_CACHE[key]

    xt_p = _kblocks(x.T.copy()).astype(NP16)
    caps = (cap0, cap1)
    in_maps = []
    core_experts = []
    for c in range(NCORES):
        exps = [slot_experts[0][c], slot_experts[1][c]]
        core_experts.append(exps)
        im = {"xt": xt_p}
        w13_bl, w2_bl, bias_cols = [], [], []
        for s, e in enumerate(exps):
            toks = tok_lists[e]
            xge = np.zeros((W, caps[s]), np.float32)
            xge[:, :len(toks)] = x[toks].T
            im[f"xg{s}"] = _kblocks(xge).astype(NP16)
            w13_bl.append(_pack_w13(w1[e] * S, w3[e] * S, MI).astype(NPW))
            w2_bl.append(_kblocks(w2[e].T @ ow.T).astype(NP16))
        for e in exps:
            bias_cols.append((S * b1[e]).reshape(MI, 128).T)
            bias_cols.append((S * b3[e]).reshape(MI, 128).T)
        sl = slice(c * SHS, (c + 1) * SHS)
        bias_cols.append((S * sb1[sl]).reshape(MS, 128).T)
        bias_cols.append((S * sb3[sl]).reshape(MS, 128).T)
        im["w13"] = np.concatenate(w13_bl, axis=1)
        im["w2ot"] = np.concatenate(w2_bl, axis=1)
        im["sw13"] = _pack_w13(sw1[sl] * S, sw3[sl] * S, MS).astype(NPW)
        im["sw2ot"] = _kblocks(sw2[:, sl].T @ ow.T).astype(NP16)
        im["bias"] = np.ascontiguousarray(
            np.concatenate(bias_cols, axis=1).astype(np.float32))
        in_maps.append(im)

    combine_w = np.zeros((B, E), np.float32)
    rows = np.arange(B)
    combine_w[rows[:, None], order] = np.take_along_axis(scores, order, axis=1)
    base = combine_w @ (b2 @ ow.T) + sb2 @ ow.T + ob

    return dict(nc=nc, caps=caps, in_maps=in_maps, tok_lists=tok_lists,
                core_experts=core_experts, combine_w=combine_w, base=base,
                s2=S * S)


def combine(p, results):
    caps, tok_lists, combine_w = p["caps"], p["tok_lists"], p["combine_w"]
    inv_s2 = 1.0 / p["s2"]
    out = p["base"].astype(np.float32).copy()
    for c in range(NCORES):
        r = results[c]
        out += r["zt"].astype(np.float32).T * inv_s2
        for s, e in enumerate(p["core_experts"][c]):
            toks = tok_lists[e]
            off = s * caps[0]
            yre = r["yr"][:, off:off + len(toks)].astype(np.float32)
            out[toks] += (combine_w[toks, e] * inv_s2)[:, None] * yre.T
    return out


def kernel(x, task_id, gate_w, w1, b1, w2, b2, w3, b3,
           sw1, sb1, sw2, sb2, sw3, sb3, ow, ob):
    global LAST_RESULTS
    p = prepare(x, task_id, gate_w, w1, b1, w2, b2, w3, b3,
                sw1, sb1, sw2, sb2, sw3, sb3, ow, ob)
    res = run_bass_kernel_spmd(
        p["nc"], p["in_maps"], core_ids=list(range(NCORES)),
        trace=TRACE, **TRACE_KW)
    LAST_RESULTS = res
    return combine(p, res.results)


# revision 13
# speedup vs baseline: 1.2094x; 1.2094x over previous
"""MoE routing kernel for 8 Trainium2 NeuronCores (Bass/Tile, SPMD).

Strategy (expert-parallel, matching the sharding hint):
  - Host computes the gate (softmax + top-2) and dispatches tokens: each of
    the 8 cores owns 2 of the 16 routed experts (slot 0 = a high-count
    expert, slot 1 = a low-count expert; per-slot capacities) and receives
    only the tokens routed to its experts.
  - The output layer (ow) folds into each expert's second matmul on the host
    (w2ot = w2[e].T @ ow.T), shrinking stage-2 work by W/OUT = 4x.
  - The shared expert is sharded over its intermediate dim (2048/8=256 rows
    per core); every core computes a partial for all 2048 tokens, with ow
    folded in.  Bias terms that commute with the output layer (b2, sb2, ob)
    are applied analytically on the host.
  - Stage-1 weights (w1/w3/sw1/sw3) are stored in fp8 e3m4 scaled by S=16;
    leaky-relu is positively homogeneous, so the descale folds into the
    host-prepared bias columns and w2ot -- zero extra device work.
  - Schedule: one global software-pipelined PE stream (expert0 m-tiles,
    expert1 m-tiles, shared chunks x m-tiles) with stage-2 matmuls trailing
    by LAG units.  Weight m-tiles stream on the SP DMA queue in consumption
    order; x/shared/bias/output DMAs ride the GpSimd queue so the ACT/DVE
    queues carry only compute.
"""
import sys

if "/opt/trn_rl_repo" not in sys.path:
    sys.path.insert(0, "/opt/trn_rl_repo")

import numpy as np
import ml_dtypes
import concourse.bass as bass
import concourse.tile as tile
from concourse import mybir
from concourse.bass_utils import run_bass_kernel_spmd

B = 2048
W = 512
E = 16
TOPK = 2
INTER = 1024
SH = 2048
OUT = 128
NCORES = 8
EPC = E // NCORES          # experts per core = 2
SHS = SH // NCORES         # shared-expert inter slice per core = 256
KW = W // 128              # k-tiles over W = 4
MI = INTER // 128          # m-tiles over INTER = 8
MS = SHS // 128            # m-tiles over shared slice = 2
F32 = mybir.dt.float32
F16 = mybir.dt.float16
E3 = mybir.dt.float8e3     # fp8 e3m4 (4 mantissa bits)
NP16 = np.float16
NPE3 = ml_dtypes.float8_e3m4

WDT = E3                   # stage-1 weight dtype knob (E3 or F16)
NPW = NPE3 if WDT == E3 else NP16
S = 16.0                   # fixed stage-1 weight scale (folds into bias/w2ot)
LAG = 2                    # stage-2 matmuls trail stage 1 by LAG units

TRACE = False
TRACE_KW = {}
LAST_RESULTS = None


def _legalize_waits(nc):
    """This container's walrus accepts at most 1 sync wait per instruction
    (2 for EventSemaphore).  Hoist excess waits emitted by the Tile
    scheduler into standalone EventSemaphore instructions."""
    for fn in nc.m.functions:
        for blk in fn.blocks:
            out = []
            changed = False
            for inst in blk.instructions:
                si = getattr(inst, "sync_info", None)
                waits = list(si.on_wait) if si is not None and si.on_wait else []
                cap = 2 if isinstance(inst, mybir.InstEventSemaphore) else 1
                if len(waits) > cap:
                    extra, keep = waits[:-cap], waits[-cap:]
                    for i in range(0, len(extra), 2):
                        out.append(mybir.InstEventSemaphore(
                            name=nc.get_next_instruction_name(),
                            engine=inst.engine,
                            ins=[], outs=[],
                            sync_info=mybir.SyncInfo(
                                on_wait=list(extra[i:i + 2]), on_update=[]),
                        ))
                    si.on_wait = keep
                    changed = True
                out.append(inst)
            if changed:
                blk.instructions = out


def _token_chunks(cap):
    chunks = []
    off = 0
    while off < cap:
        sz = min(512, cap - off)
        chunks.append((off, sz))
        off += sz
    return chunks


def _build_nc(cap0, cap1, legalize=True):
    """SPMD Bass program for per-slot token capacities (multiples of 16)."""
    nc = bass.Bass("TRN2", target_bir_lowering=False, debug=False)
    caps = (cap0, cap1)

    def din(name, f, dt):
        return nc.dram_tensor(name, [128, f], dt, kind="ExternalInput").ap()

    xg0 = din("xg0", KW * cap0, F16)     # slot-0 gathered tokens, k-blocks
    xg1 = din("xg1", KW * cap1, F16)
    # per (slot, m): [w1 m-tile k0..3 | w3 m-tile k0..3], each [128,128]
    w13 = din("w13", EPC * MI * 2 * KW * 128, WDT)
    w2ot = din("w2ot", EPC * MI * OUT, F16)
    xt = din("xt", KW * B, F16)          # x.T k-blocks (shared expert)
    sw13 = din("sw13", MS * 2 * KW * 128, WDT)
    sw2ot = din("sw2ot", MS * OUT, F16)
    bias = din("bias", EPC * 2 * MI + 2 * MS, F32)

    yr = nc.dram_tensor("yr", [128, cap0 + cap1], F16, kind="ExternalOutput").ap()
    zt = nc.dram_tensor("zt", [128, B], F16, kind="ExternalOutput").ap()

    LR = mybir.ActivationFunctionType.Lrelu

    EW = MI * 2 * KW * 128      # w13 cols per expert slot

    with tile.TileContext(nc) as tc:
        with tc.tile_pool(name="wts", bufs=1) as wts, \
             tc.tile_pool(name="work", bufs=3) as work, \
             tc.tile_pool(name="hts", bufs=4) as hts, \
             tc.tile_pool(name="outs", bufs=2) as outs, \
             tc.tile_pool(name="ps", bufs=2, space="PSUM") as ps:

            # The DMA fabric round-robins packets across all in-flight
            # descriptors, so a transfer completes only when everything
            # issued alongside it does.  Stagger issues across the unit
            # stream: keep in flight only what the PE needs next.
            bias_t = wts.tile([128, bias.shape[1]], F32, tag="bias")
            nc.scalar.dma_start(bias_t[:], bias[:])

            # first-MM gate: xg0 + w13 m0/m1 only
            xg_ts = [None, None]
            xg_ts[0] = wts.tile([128, KW * cap0], F16, tag="xg0", name="xg0t")
            nc.sync.dma_start(xg_ts[0][:], xg0[:])
            w13e = []
            for s in range(EPC):
                w13e.append(wts.tile([128, EW], WDT, tag=f"w13e{s}",
                                     name=f"w13e{s}t"))
            MW = 2 * KW * 128           # cols per m-tile
            nc.sync.dma_start(w13e[0][:, 0:2 * MW], w13[:, 0:2 * MW])

            w2ot_ts = []
            for s in range(EPC):
                w2ot_ts.append(wts.tile([128, MI * OUT], F16, tag=f"w2ot{s}",
                                        name=f"w2ot{s}t"))
            sw13_t = wts.tile([128, MS * 2 * KW * 128], WDT, tag="sw13")
            sw2ot_t = wts.tile([128, MS * OUT], F16, tag="sw2ot")
            xt_t = wts.tile([128, KW * B], F16, tag="xt")

            def w13_dma(s, mlo, mhi):
                def f():
                    nc.sync.dma_start(
                        w13e[s][:, mlo * MW:mhi * MW],
                        w13[:, s * EW + mlo * MW:s * EW + mhi * MW])
                return f

            def xg1_dma():
                xg_ts[1] = wts.tile([128, KW * cap1], F16, tag="xg1",
                                    name="xg1t")
                nc.sync.dma_start(xg_ts[1][:], xg1[:])

            def xt_dma(klo, khi):
                def f():
                    nc.scalar.dma_start(xt_t[:, klo * B:khi * B],
                                        xt[:, klo * B:khi * B])
                return f

            def b_col(col):
                return bias_t[:, col:col + 1]

            # ---- PE warmup: dummy matmuls ramp the clock while DMAs land --
            wz = wts.tile([128, 256], F16, tag="wz")
            nc.gpsimd.memset(wz[:], 0.0)
            pwarm = ps.tile([128, 128], F32, tag="py", bufs=2, name="pwarm")
            for _ in range(20):
                nc.tensor.matmul(pwarm[:], wz[:, 0:128], wz[:, 128:256],
                                 start=True, stop=True)

            # ---- unified software-pipelined compute stream ----
            pend = []

            def emit_stage2(f):
                f()

            def unit_stage1(csz, lhs_tile, lhs_col0, rhs_tile, rhs_cols,
                            b1c, b3c):
                """Emit stage-1 matmuls + act/mul for one (unit, m-tile)."""
                p1 = ps.tile([128, csz], F32, tag="p1", bufs=2)
                p3 = ps.tile([128, csz], F32, tag="p3", bufs=2)
                for k in range(KW):
                    lhs = lhs_tile[:, lhs_col0 + k * 128:lhs_col0 + (k + 1) * 128]
                    rhs = rhs_tile[:, rhs_cols[k]:rhs_cols[k] + csz]
                    nc.tensor.matmul(p1[:], lhs, rhs,
                                     start=(k == 0), stop=(k == KW - 1))
                for k in range(KW):
                    lhs = lhs_tile[:, lhs_col0 + (KW + k) * 128:
                                   lhs_col0 + (KW + k + 1) * 128]
                    rhs = rhs_tile[:, rhs_cols[k]:rhs_cols[k] + csz]
                    nc.tensor.matmul(p3[:], lhs, rhs,
                                     start=(k == 0), stop=(k == KW - 1))
                a = work.tile([128, csz], F16, tag="act_a")
                nc.scalar.activation(a[:], p1[:], LR, bias=b1c, alpha=0.01)
                t3 = work.tile([128, csz], F16, tag="act_b")
                nc.vector.tensor_scalar_add(t3[:], p3[:], b3c)
                h = hts.tile([128, csz], F16, tag="h")
                nc.vector.tensor_mul(h[:], a[:], t3[:])
                return h

            # unit order: slot-0 expert, slot-1 expert, shared (small tail)
            units = []
            for (c0, csz) in _token_chunks(cap0):
                for m in range(MI):
                    units.append(("r", 0, c0, csz, m))
            for (c0, csz) in _token_chunks(cap1):
                for m in range(MI):
                    units.append(("r", 1, c0, csz, m))
            sh_chunks = [(0, 512), (512, 512), (1024, 512),
                         (1536, 256), (1792, 256)]
            for (c0, csz) in sh_chunks:
                for m in range(MS):
                    units.append(("s", 0, c0, csz, m))

            # just-in-time DMA issue: hooks keyed by unit index
            def w2ot_dma(s):
                def f():
                    nc.sync.dma_start(
                        w2ot_ts[s][:],
                        w2ot[:, s * MI * OUT:(s + 1) * MI * OUT])
                return f

            def sw_dma():
                nc.sync.dma_start(sw13_t[:], sw13[:])
                nc.sync.dma_start(sw2ot_t[:], sw2ot[:])

            hooks = {
                0: w13_dma(0, 2, 4),
                1: w2ot_dma(0),
                2: w13_dma(0, 4, 6),
                3: lambda: (xg1_dma(), w13_dma(1, 0, 2)()),
                4: w13_dma(0, 6, 8),
                5: xt_dma(0, 2),
                6: w13_dma(1, 2, 5),
                7: xt_dma(2, 4),
                8: w13_dma(1, 5, 8),      # e1 m0
                9: w2ot_dma(1),           # e1 m1
                11: sw_dma,               # e1 m3
            }
            py_tiles = {}
            for ui, u in enumerate(units):
                if ui in hooks:
                    hooks[ui]()
                kind, s, c0, csz, m = u
                if kind == "r":
                    rhs_cols = [k * caps[s] + c0 for k in range(KW)]
                    h = unit_stage1(csz, w13e[s], m * 2 * KW * 128,
                                    xg_ts[s], rhs_cols,
                                    b_col(s * 2 * MI + m),
                                    b_col(s * 2 * MI + MI + m))
                    key = ("r", s, c0)
                    if key not in py_tiles:
                        py_tiles[key] = ps.tile([128, csz], F32, tag="py", bufs=2, name="py")
                    py = py_tiles[key]
                    w2t = w2ot_ts[s]
                    lhs2 = w2t[:, m * OUT:(m + 1) * OUT]
                    last = (m == MI - 1)

                    def mk(py=py, lhs2=lhs2, h=h, m=m, last=last, s=s,
                           c0=c0, csz=csz):
                        def f():
                            nc.tensor.matmul(py[:], lhs2, h[:],
                                             start=(m == 0), stop=last)
                            if last:
                                yo = outs.tile([128, csz], F16, tag="yo")
                                nc.vector.tensor_copy(yo[:], py[:])
                                off = s * cap0 + c0
                                nc.scalar.dma_start(
                                    yr[:, off:off + csz], yo[:])
                        return f
                    pend.append(mk())
                else:
                    rhs_cols = [k * B + c0 for k in range(KW)]
                    h = unit_stage1(csz, sw13_t, m * 2 * KW * 128,
                                    xt_t, rhs_cols,
                                    b_col(EPC * 2 * MI + m),
                                    b_col(EPC * 2 * MI + MS + m))
                    key = ("s", 0, c0)
                    if key not in py_tiles:
                        py_tiles[key] = ps.tile([128, csz], F32, tag="pz", bufs=2, name="pz")
                    pz = py_tiles[key]
                    lhs2 = sw2ot_t[:, m * OUT:(m + 1) * OUT]
                    last = (m == MS - 1)

                    def mk(pz=pz, lhs2=lhs2, h=h, m=m, last=last,
                           c0=c0, csz=csz):
                        def f():
                            nc.tensor.matmul(pz[:], lhs2, h[:],
                                             start=(m == 0), stop=last)
                            if last:
                                zo = outs.tile([128, csz], F16, tag="zo")
                                nc.vector.tensor_copy(zo[:], pz[:])
                                nc.scalar.dma_start(
                                    zt[:, c0:c0 + csz], zo[:])
                        return f
                    pend.append(mk())
                if len(pend) > LAG:
                    emit_stage2(pend.pop(0))
            for f in pend:
                emit_stage2(f)

    if legalize:
        _legalize_waits(nc)
    return nc


_NC_CACHE = {}


def _kblocks(mat):
    """[Ktot, F] -> [128, (Ktot/128)*F], col block k = mat[128k:128(k+1), :]."""
    ktot, f = mat.shape
    assert ktot % 128 == 0
    return np.ascontiguousarray(
        mat.reshape(ktot // 128, 128, f).transpose(1, 0, 2).reshape(128, -1))


def _pack_w13(w1e, w3e, mtiles):
    """w1e/w3e: [I, W] fp32 (already scaled).  Returns [128, mtiles*2*KW*128]
    with per-m layout [w1 k0..3 | w3 k0..3]."""
    blocks = []
    w1t = w1e.T.reshape(KW, 128, mtiles * 128)   # [k, 128, I]
    w3t = w3e.T.reshape(KW, 128, mtiles * 128)
    for m in range(mtiles):
        for wt in (w1t, w3t):
            for k in range(KW):
                blocks.append(wt[k][:, m * 128:(m + 1) * 128])
    return np.ascontiguousarray(np.concatenate(blocks, axis=1))


def prepare(x, task_id, gate_w, w1, b1, w2, b2, w3, b3,
            sw1, sb1, sw2, sb2, sw3, sb3, ow, ob):
    x = np.asarray(x, np.float32)
    f32 = lambda a: np.asarray(a, np.float32)
    gate_w, w1, b1, w2, b2, w3, b3 = map(f32, (gate_w, w1, b1, w2, b2, w3, b3))
    sw1, sb1, sw2, sb2, sw3, sb3, ow, ob = map(
        f32, (sw1, sb1, sw2, sb2, sw3, sb3, ow, ob))

    # ---- host gate: softmax + top-2 ----
    logits = x @ gate_w.T
    logits -= logits.max(axis=1, keepdims=True)
    ex = np.exp(logits)
    scores = ex / ex.sum(axis=1, keepdims=True)
    order = np.argsort(-scores, axis=1, kind="stable")[:, :TOPK]

    tok_lists = [np.nonzero((order == e).any(axis=1))[0] for e in range(E)]
    counts = np.array([len(t) for t in tok_lists])
    ranked = np.argsort(-counts, kind="stable")
    slot_experts = [list(ranked[:NCORES]), list(ranked[NCORES:][::-1])]
    rup = lambda n: max(128, -(-n // 16) * 16)
    cap0 = rup(max(counts[e] for e in slot_experts[0]))
    cap1 = rup(max(counts[e] for e in slot_experts[1]))

    key = (cap0, cap1)
    if key not in _NC_CACHE:
        _NC_CACHE[key] = _build_nc(cap0, cap1)
    nc = _NC_CACHE[key]

    xt_p = _kblocks(x.T.copy()).astype(NP16)
    caps = (cap0, cap1)
    in_maps = []
    core_experts = []
    for c in range(NCORES):
        exps = [slot_experts[0][c], slot_experts[1][c]]
        core_experts.append(exps)
        im = {"xt": xt_p}
        w13_bl, w2_bl, bias_cols = [], [], []
        for s, e in enumerate(exps):
            toks = tok_lists[e]
            xge = np.zeros((W, caps[s]), np.float32)
            xge[:, :len(toks)] = x[toks].T
            im[f"xg{s}"] = _kblocks(xge).astype(NP16)
            w13_bl.append(_pack_w13(w1[e] * S, w3[e] * S, MI).astype(NPW))
            w2_bl.append(_kblocks(w2[e].T @ ow.T).astype(NP16))
        for e in exps:
            bias_cols.append((S * b1[e]).reshape(MI, 128).T)
            bias_cols.append((S * b3[e]).reshape(MI, 128).T)
        sl = slice(c * SHS, (c + 1) * SHS)
        bias_cols.append((S * sb1[sl]).reshape(MS, 128).T)
        bias_cols.append((S * sb3[sl]).reshape(MS, 128).T)
        im["w13"] = np.concatenate(w13_bl, axis=1)
        im["w2ot"] = np.concatenate(w2_bl, axis=1)
        im["sw13"] = _pack_w13(sw1[sl] * S, sw3[sl] * S, MS).astype(NPW)
        im["sw2ot"] = _kblocks(sw2[:, sl].T @ ow.T).astype(NP16)
        im["bias"] = np.ascontiguousarray(
            np.concatenate(bias_cols, axis=1).astype(np.float32))
        in_maps.append(im)

    combine_w = np.zeros((B, E), np.float32)
    rows = np.arange(B)
    combine_w[rows[:, None], order] = np.take_along_axis(scores, order, axis=1)
    base = combine_w @ (b2 @ ow.T) + sb2 @ ow.T + ob

    return dict(nc=nc, caps=caps, in_maps=in_maps, tok_lists=tok_lists,
                core_experts=core_experts, combine_w=combine_w, base=base,
                s2=S * S)


def combine(p, results):
    caps, tok_lists, combine_w = p["caps"], p["tok_lists"], p["combine_w"]
    inv_s2 = 1.0 / p["s2"]
    out = p["base"].astype(np.float32).copy()
    for c in range(NCORES):
        r = results[c]
        out += r["zt"].astype(np.float32).T * inv_s2
        for s, e in enumerate(p["core_experts"][c]):
            toks = tok_lists[e]
            off = s * caps[0]
            yre = r["yr"][:, off:off + len(toks)].astype(np.float32)
            out[toks] += (combine_w[toks, e] * inv_s2)[:, None] * yre.T
    return out


def kernel(x, task_id, gate_w, w1, b1, w2, b2, w3, b3,
           sw1, sb1, sw2, sb2, sw3, sb3, ow, ob):
    global LAST_RESULTS
    p = prepare(x, task_id, gate_w, w1, b1, w2, b2, w3, b3,
                sw1, sb1, sw2, sb2, sw3, sb3, ow, ob)
    res = run_bass_kernel_spmd(
        p["nc"], p["in_maps"], core_ids=list(range(NCORES)),
        trace=TRACE, **TRACE_KW)
    LAST_RESULTS = res
    return combine(p, res.results)
